# revision 30
# baseline (speedup 1.0000x reference)
"""Trainium2 Bass kernel: masked attention-energy softmax.

Computes, for each batch row b:
    energy[b, t] = v . (W @ q[b, t] + bias)          (== q[b, t] . (W^T v) + bias . v)
    out[b]      = softmax(mask(energy[b]), axis=t)   with t >= len[b] masked to -1e10

Strategy
--------
* Pure data parallel over 8 NeuronCores: 8 batch rows per core.  W/b/v are
  folded on host into u = W^T v (the bias.v constant shifts every energy in a
  row equally, so it cancels in softmax and is dropped).
* The per-token energy is a dot product E[b,t] = q[b,t,:] . u -- TensorEngine
  work.  The PE contracts along partitions, so q is pre-transposed on host
  with h on the partition axis.  This is a memory-bound problem, so the h
  components are split by |u_h| into mixed precision: the top 128 stream as
  bf16, the bottom 128 (6.8% of sum u^2) as fp8 e3m4 -- 6.3 MB/core instead
  of 16.8 fp32.  Measured softmax error 4.9e-3 on the reference data (gate
  2e-2).  Both stationaries are scaled by 64 so fp8 u values clear the
  denormal floor; exp() compensates with scale=1/64.
* Each batch's energies land on PSUM partition b via a block-diagonal
  stationary: column b holds u's half, zeros elsewhere (PE column strips
  force output partition bases to 0/32/64/96, so an M=8 stationary is how
  all 8 rows share one [8, 2048] PSUM tile).  64 matmuls (8 b x 2 halves x
  4 n-tiles of 512) accumulate E per bank.
* The ragged mask nm[b,t] = -1e10 * (t >= len[b]) is built once on DVE from a
  host iota and the lens vector, then added in PSUM by 4 more matmuls with an
  8x8 identity stationary -- no full-width DVE add.
* Tail, all on [8, 2048]: exp(E/64) + per-row accumulate on ScalarE straight
  from PSUM, reciprocal + scale on DVE, one 64 KB store.  No max-subtraction
  (u has unit norm so E ~ N(0,1)); no cross-partition reduce (each row owns
  one partition), no gpsimd.
"""

import numpy as np

B, T, H = 64, 2048, 256
NCORES = 8
NB = B // NCORES  # batches per core
NT = 4  # token tiles of 512 (one PSUM bank each)
TT = T // NT
NEG = -1.0e10
USCALE = 64.0  # stationary pre-scale so fp8 u values stay normal
QBUFS = 10  # q tile pool depth (per-batch tiles: 4 KB + 2 KB per partition)

_CACHE = {}


def _build_nc(reps=1):
    """Build the per-core Bass program.  reps>1 statically unrolls the whole
    computation for benchmarking (marginal per-rep wall time isolates HW
    execution time from axon dispatch overhead); the graded path uses reps=1.
    """
    from contextlib import ExitStack

    import concourse.bacc as bacc
    import concourse.tile as tile
    from concourse import mybir

    f32 = mybir.dt.float32
    bf16 = mybir.dt.bfloat16
    f8 = mybir.dt.float8e3
    nc = bacc.Bacc("TRN2", target_bir_lowering=False, debug=False)

    # batches ride in groups so every DMA moves ~1 MB (8 KB per partition):
    # bf16 in pairs, fp8 in quads
    qbf_d = nc.dram_tensor("qbf", [NB // 2, 128, 2, T], bf16, kind="ExternalInput").ap()
    qf8_d = nc.dram_tensor("qf8", [NB // 4, 128, 4, T], f8, kind="ExternalInput").ap()
    ubf_d = nc.dram_tensor("ubf", [128, NB, NB], bf16, kind="ExternalInput").ap()
    uf8_d = nc.dram_tensor("uf8", [128, NB, NB], f8, kind="ExternalInput").ap()
    id8_d = nc.dram_tensor("id8", [NB, NB], bf16, kind="ExternalInput").ap()
    iota_d = nc.dram_tensor("iota", [NB, T], f32, kind="ExternalInput").ap()
    lens_d = nc.dram_tensor("lens", [NB, 1], f32, kind="ExternalInput").ap()
    out_d = nc.dram_tensor("out", [NB, T], f32, kind="ExternalOutput").ap()

    with tile.TileContext(nc) as tc, ExitStack() as ctx:
        singles = ctx.enter_context(tc.tile_pool(name="singles", bufs=1))
        qbpool = ctx.enter_context(tc.tile_pool(name="qbpool", bufs=QBUFS))
        qfpool = ctx.enter_context(tc.tile_pool(name="qfpool", bufs=QBUFS))
        ppool = ctx.enter_context(tc.tile_pool(name="ppool", bufs=2, space="PSUM"))
        spool = ctx.enter_context(tc.tile_pool(name="spool", bufs=2))

        ubf = singles.tile([128, NB, NB], bf16)
        nc.sync.dma_start(out=ubf, in_=ubf_d)
        uf8 = singles.tile([128, NB, NB], f8)
        nc.sync.dma_start(out=uf8, in_=uf8_d)
        id8 = singles.tile([NB, NB], bf16)
        nc.sync.dma_start(out=id8, in_=id8_d)
        iota_f = singles.tile([NB, T], f32)
        nc.sync.dma_start(out=iota_f, in_=iota_d)
        lens_sb = singles.tile([NB, 1], f32)
        nc.sync.dma_start(out=lens_sb, in_=lens_d)

        # nm[b, t] = NEG * (t >= len[b]); bf16 so it can ride a PE matmul
        nm = singles.tile([NB, T], bf16)
        nc.vector.tensor_scalar(
            out=nm,
            in0=iota_f,
            scalar1=lens_sb[:, 0:1],
            scalar2=NEG,
            op0=mybir.AluOpType.is_ge,
            op1=mybir.AluOpType.mult,
        )

        for _rep in range(reps):
            qbtiles, qftiles = [], []
            for b in range(NB):
                if b % 2 == 0:
                    qb = qbpool.tile([128, 2, T], bf16, tag="qb")
                    nc.sync.dma_start(out=qb, in_=qbf_d[b // 2])
                    qbtiles.append(qb)
                if b % 4 == 0:
                    qf = qfpool.tile([128, 4, T], f8, tag="qf")
                    nc.sync.dma_start(out=qf, in_=qf8_d[b // 4])
                    qftiles.append(qf)

            # USCALE * E[b, t] = sum_h q[b, t, h] * u[h], batch b on PSUM row b
            ep = ppool.tile([NB, T], f32, tag="ep")
            for b in range(NB):
                for nt in range(NT):
                    sl = slice(nt * TT, (nt + 1) * TT)
                    nc.tensor.matmul(
                        ep[:, sl],
                        ubf[:, b, :],
                        qbtiles[b // 2][:, b % 2, sl],
                        start=(b == 0),
                        stop=False,
                    )
                    nc.tensor.matmul(
                        ep[:, sl],
                        uf8[:, b, :],
                        qftiles[b // 4][:, b % 4, sl],
                        start=False,
                        stop=False,
                    )
            # E += nm  (identity stationary: out[m, t] += sum_k id8[k, m] nm[k, t])
            for nt in range(NT):
                nc.tensor.matmul(
                    ep[:, nt * TT : (nt + 1) * TT],
                    id8,
                    nm[:, nt * TT : (nt + 1) * TT],
                    start=False,
                    stop=True,
                )

            # expE[b, :] = exp(E[b, :] / USCALE), acc[b] = sum_t expE[b, t]
            # (masked slots hold ~ -1e10, exp -> 0 exactly)
            expE = spool.tile([NB, T], f32, tag="expE")
            acc = spool.tile([NB, 1], f32, tag="acc")
            nc.scalar.activation(
                out=expE,
                in_=ep,
                func=mybir.ActivationFunctionType.Exp,
                scale=1.0 / USCALE,
                accum_out=acc,
            )
            recip = spool.tile([NB, 1], f32, tag="recip")
            nc.vector.reciprocal(recip, acc)
            probs = spool.tile([NB, T], f32, tag="probs")
            nc.vector.tensor_scalar_mul(probs, expE, recip[:, 0:1])
            nc.sync.dma_start(out=out_d, in_=probs)

    nc.compile()
    return nc


def _prep_inputs(questions, questions_lens, W, b, v):
    import ml_dtypes

    bf16 = ml_dtypes.bfloat16
    f8 = ml_dtypes.float8_e3m4
    q = np.asarray(questions, dtype=np.float32)
    lens = np.asarray(questions_lens)
    W = np.asarray(W, dtype=np.float32)
    v = np.asarray(v, dtype=np.float32)
    u = (W.T @ v).astype(np.float32)

    # split h by |u_h|: top 128 ride bf16, bottom 128 (a few % of energy
    # variance) ride fp8 e3m4
    order = np.argsort(-np.abs(u))
    top, bot = order[:128], order[128:]

    def blockdiag(vals, dt):
        ust = np.zeros((128, NB, NB), dtype=dt)
        cast = (vals * USCALE).astype(dt)
        for bb in range(NB):
            ust[:, bb, bb] = cast
        return ust

    ubf = blockdiag(u[top], bf16)
    uf8 = blockdiag(u[bot], f8)
    id8 = np.eye(NB, dtype=bf16)
    iota = np.broadcast_to(np.arange(T, dtype=np.float32), (NB, T)).copy()
    lens_f = lens.astype(np.float32).reshape(B, 1)

    in_maps = []
    for c in range(NCORES):
        qc = q[c * NB : (c + 1) * NB]  # [NB, T, H]
        # [group, p, b_in_group, t]: each partition's group slice is 8 KB
        # contiguous in HBM, every DMA moves ~1 MB
        qbf = (
            np.ascontiguousarray(qc[:, :, top].transpose(0, 2, 1))
            .astype(bf16)
            .reshape(NB // 2, 2, 128, T)
            .transpose(0, 2, 1, 3)
            .copy()
        )
        qf8 = (
            np.ascontiguousarray(qc[:, :, bot].transpose(0, 2, 1))
            .astype(f8)
            .reshape(NB // 4, 4, 128, T)
            .transpose(0, 2, 1, 3)
            .copy()
        )
        in_maps.append(
            {
                "qbf": qbf,
                "qf8": qf8,
                "ubf": ubf,
                "uf8": uf8,
                "id8": id8,
                "iota": iota,
                "lens": lens_f[c * NB : (c + 1) * NB],
            }
        )
    return in_maps


def _get_runner(reps=1):
    """Build (once per reps) a persistent sharded-jit runner over the 8 cores.

    Mirrors concourse.bass2jax.run_bass_via_pjrt's multi-core path, but caches
    the jitted executable so repeated calls skip retrace/recompile.  Used for
    benchmarking; the graded kernel() path goes through run_bass_kernel_spmd.
    """
    key = ("runner", reps)
    if key in _CACHE:
        return _CACHE[key]

    import jax
    from jax.sharding import Mesh, PartitionSpec
    from jax.experimental.shard_map import shard_map

    import concourse.mybir as mybir
    from concourse.bass2jax import (
        _bass_exec_p,
        install_neuronx_cc_hook,
        partition_id_tensor,
    )

    nc = _build_nc(reps)
    install_neuronx_cc_hook()

    partition_name = nc.partition_id_tensor.name if nc.partition_id_tensor else None
    in_names, out_names, out_avals, zero_outs = [], [], [], []
    for alloc in nc.m.functions[0].allocations:
        if not isinstance(alloc, mybir.MemoryLocationSet):
            continue
        name = alloc.memorylocations[0].name
        if alloc.kind == "ExternalInput":
            if name != partition_name:
                in_names.append(name)
        elif alloc.kind == "ExternalOutput":
            out_names.append(name)
            shape = tuple(alloc.tensor_shape)
            dtype = mybir.dt.np(alloc.dtype)
            out_avals.append(jax.core.ShapedArray(shape, dtype))
            zero_outs.append(np.zeros(shape, dtype))
    n_params = len(in_names)
    all_in_names = list(in_names) + list(out_names)
    if partition_name is not None:
        all_in_names.append(partition_name)

    def _body(*args):
        operands = list(args)
        if partition_name is not None:
            operands.append(partition_id_tensor())
        outs = _bass_exec_p.bind(
            *operands,
            out_avals=tuple(out_avals),
            in_names=tuple(all_in_names),
            out_names=tuple(out_names),
            lowering_input_output_aliases=(),
            sim_require_finite=True,
            sim_require_nnan=True,
            nc=nc,
        )
        return tuple(outs)

    devices = jax.devices()[:NCORES]
    mesh = Mesh(np.asarray(devices), ("core",))
    n_outs = len(out_names)
    in_specs = (PartitionSpec("core"),) * (n_params + n_outs)
    out_specs = (PartitionSpec("core"),) * n_outs
    sharded = jax.jit(
        shard_map(
            _body, mesh=mesh, in_specs=in_specs, out_specs=out_specs, check_rep=False
        ),
        donate_argnums=tuple(range(n_params, n_params + n_outs)),
        keep_unused=True,
    )

    def run(in_maps):
        concat_in = [
            np.concatenate([np.asarray(m[name]) for m in in_maps], axis=0)
            for name in in_names
        ]
        concat_zeros = [
            np.zeros((NCORES * z.shape[0], *z.shape[1:]), z.dtype) for z in zero_outs
        ]
        out_arrs = sharded(*concat_in, *concat_zeros)
        return {
            name: np.asarray(out_arrs[i]).reshape(NCORES * out_avals[i].shape[0], *out_avals[i].shape[1:])
            for i, name in enumerate(out_names)
        }

    _CACHE[("parts", reps)] = dict(
        sharded=sharded,
        in_names=in_names,
        out_names=out_names,
        out_avals=out_avals,
        zero_outs=zero_outs,
        mesh=mesh,
    )
    _CACHE[key] = run
    return run


def kernel(questions, questions_lens, W, b, v):
    """Full-input entry point: shards across the 8 NeuronCores, runs the Bass
    kernel via run_bass_kernel_spmd, gathers the full [64, 2048] output."""
    from concourse.bass_utils import run_bass_kernel_spmd

    if "nc" not in _CACHE:
        _CACHE["nc"] = _build_nc()
    in_maps = _prep_inputs(questions, questions_lens, W, b, v)
    res = run_bass_kernel_spmd(_CACHE["nc"], in_maps, list(range(NCORES)))
    return np.concatenate([r["out"] for r in res.results], axis=0)


# revision 31
# speedup vs baseline: 1.0766x; 1.0766x over previous
"""Trainium2 Bass kernel: masked attention-energy softmax.

Computes, for each batch row b:
    energy[b, t] = v . (W @ q[b, t] + bias)          (== q[b, t] . (W^T v) + bias . v)
    out[b]      = softmax(mask(energy[b]), axis=t)   with t >= len[b] masked to -1e10

Strategy
--------
* Pure data parallel over 8 NeuronCores: 8 batch rows per core.  W/b/v are
  folded on host into u = W^T v (the bias.v constant shifts every energy in a
  row equally, so it cancels in softmax and is dropped).
* The per-token energy is a dot product E[b,t] = q[b,t,:] . u -- TensorEngine
  work.  The PE contracts along partitions, so q is pre-transposed on host
  with h on the partition axis.  This is a memory-bound problem, so the h
  components are split by |u_h| into mixed precision: the top 128 stream as
  bf16, the bottom 128 (6.8% of sum u^2) as fp8 e3m4 -- 6.3 MB/core instead
  of 16.8 fp32.  Measured softmax error 4.9e-3 on the reference data (gate
  2e-2).  Both stationaries are scaled by 64 so fp8 u values clear the
  denormal floor; exp() compensates with scale=1/64.
* Each batch's energies land on PSUM partition b via a block-diagonal
  stationary: column b holds u's half, zeros elsewhere (PE column strips
  force output partition bases to 0/32/64/96, so an M=8 stationary is how
  all 8 rows share one [8, 2048] PSUM tile).  64 matmuls (8 b x 2 halves x
  4 n-tiles of 512) accumulate E per bank.
* The ragged mask nm[b,t] = -1e10 * (t >= len[b]) is built once on DVE from a
  host iota and the lens vector, then added in PSUM by 4 more matmuls with an
  8x8 identity stationary -- no full-width DVE add.
* Tail, all on [8, 2048]: exp(E/64) + per-row accumulate on ScalarE straight
  from PSUM, reciprocal + scale on DVE, one 64 KB store.  No max-subtraction
  (u has unit norm so E ~ N(0,1)); no cross-partition reduce (each row owns
  one partition), no gpsimd.
"""

import numpy as np

B, T, H = 64, 2048, 256
NCORES = 8
NB = B // NCORES  # batches per core
NT = 4  # token tiles of 512 (one PSUM bank each)
TT = T // NT
NEG = -1.0e10
USCALE = 64.0  # stationary pre-scale so fp8 u values stay normal
QBUFS = 10  # q tile pool depth (per-batch tiles: 4 KB + 2 KB per partition)

_CACHE = {}


def _build_nc(reps=1):
    """Build the per-core Bass program.  reps>1 statically unrolls the whole
    computation for benchmarking (marginal per-rep wall time isolates HW
    execution time from axon dispatch overhead); the graded path uses reps=1.
    """
    from contextlib import ExitStack

    import concourse.bacc as bacc
    import concourse.tile as tile
    from concourse import mybir

    f32 = mybir.dt.float32
    bf16 = mybir.dt.bfloat16
    f8 = mybir.dt.float8e3
    nc = bacc.Bacc("TRN2", target_bir_lowering=False, debug=False)

    qbf_d = nc.dram_tensor("qbf", [NB, 128, T], bf16, kind="ExternalInput").ap()
    # fp8 batches ride in pairs: one 512 KB DMA instead of two 256 KB ones
    qf8_d = nc.dram_tensor("qf8", [NB // 2, 128, 2, T], f8, kind="ExternalInput").ap()
    ubf_d = nc.dram_tensor("ubf", [128, NB, NB], bf16, kind="ExternalInput").ap()
    uf8_d = nc.dram_tensor("uf8", [128, NB, NB], f8, kind="ExternalInput").ap()
    id8_d = nc.dram_tensor("id8", [NB, NB], bf16, kind="ExternalInput").ap()
    iota_d = nc.dram_tensor("iota", [NB, T], f32, kind="ExternalInput").ap()
    lens_d = nc.dram_tensor("lens", [NB, 1], f32, kind="ExternalInput").ap()
    out_d = nc.dram_tensor("out", [NB, T], f32, kind="ExternalOutput").ap()

    with tile.TileContext(nc) as tc, ExitStack() as ctx:
        singles = ctx.enter_context(tc.tile_pool(name="singles", bufs=1))
        qbpool = ctx.enter_context(tc.tile_pool(name="qbpool", bufs=QBUFS))
        qfpool = ctx.enter_context(tc.tile_pool(name="qfpool", bufs=QBUFS))
        ppool = ctx.enter_context(tc.tile_pool(name="ppool", bufs=2, space="PSUM"))
        spool = ctx.enter_context(tc.tile_pool(name="spool", bufs=2))

        ubf = singles.tile([128, NB, NB], bf16)
        nc.sync.dma_start(out=ubf, in_=ubf_d)
        uf8 = singles.tile([128, NB, NB], f8)
        nc.sync.dma_start(out=uf8, in_=uf8_d)
        id8 = singles.tile([NB, NB], bf16)
        nc.sync.dma_start(out=id8, in_=id8_d)
        iota_f = singles.tile([NB, T], f32)
        nc.sync.dma_start(out=iota_f, in_=iota_d)
        lens_sb = singles.tile([NB, 1], f32)
        nc.sync.dma_start(out=lens_sb, in_=lens_d)

        # nm[b, t] = NEG * (t >= len[b]); bf16 so it can ride a PE matmul
        nm = singles.tile([NB, T], bf16)
        nc.vector.tensor_scalar(
            out=nm,
            in0=iota_f,
            scalar1=lens_sb[:, 0:1],
            scalar2=NEG,
            op0=mybir.AluOpType.is_ge,
            op1=mybir.AluOpType.mult,
        )

        for _rep in range(reps):
            qbtiles, qftiles = [], []
            for b in range(NB):
                qb = qbpool.tile([128, T], bf16, tag="qb")
                nc.sync.dma_start(out=qb, in_=qbf_d[b])
                qbtiles.append(qb)
                if b % 2 == 0:
                    qf = qfpool.tile([128, 2, T], f8, tag="qf")
                    nc.sync.dma_start(out=qf, in_=qf8_d[b // 2])
                    qftiles.append(qf)

            # USCALE * E[b, t] = sum_h q[b, t, h] * u[h], batch b on PSUM row b
            ep = ppool.tile([NB, T], f32, tag="ep")
            for b in range(NB):
                for nt in range(NT):
                    sl = slice(nt * TT, (nt + 1) * TT)
                    nc.tensor.matmul(
                        ep[:, sl],
                        ubf[:, b, :],
                        qbtiles[b][:, sl],
                        start=(b == 0),
                        stop=False,
                    )
                    nc.tensor.matmul(
                        ep[:, sl],
                        uf8[:, b, :],
                        qftiles[b // 2][:, b % 2, sl],
                        start=False,
                        stop=False,
                    )
            # E += nm  (identity stationary: out[m, t] += sum_k id8[k, m] nm[k, t])
            for nt in range(NT):
                nc.tensor.matmul(
                    ep[:, nt * TT : (nt + 1) * TT],
                    id8,
                    nm[:, nt * TT : (nt + 1) * TT],
                    start=False,
                    stop=True,
                )

            # expE[b, :] = exp(E[b, :] / USCALE), acc[b] = sum_t expE[b, t]
            # (masked slots hold ~ -1e10, exp -> 0 exactly)
            expE = spool.tile([NB, T], f32, tag="expE")
            acc = spool.tile([NB, 1], f32, tag="acc")
            nc.scalar.activation(
                out=expE,
                in_=ep,
                func=mybir.ActivationFunctionType.Exp,
                scale=1.0 / USCALE,
                accum_out=acc,
            )
            recip = spool.tile([NB, 1], f32, tag="recip")
            nc.vector.reciprocal(recip, acc)
            probs = spool.tile([NB, T], f32, tag="probs")
            nc.vector.tensor_scalar_mul(probs, expE, recip[:, 0:1])
            nc.sync.dma_start(out=out_d, in_=probs)

    nc.compile()
    return nc


def _prep_inputs(questions, questions_lens, W, b, v):
    import ml_dtypes

    bf16 = ml_dtypes.bfloat16
    f8 = ml_dtypes.float8_e3m4
    q = np.asarray(questions, dtype=np.float32)
    lens = np.asarray(questions_lens)
    W = np.asarray(W, dtype=np.float32)
    v = np.asarray(v, dtype=np.float32)
    u = (W.T @ v).astype(np.float32)

    # split h by |u_h|: top 128 ride bf16, bottom 128 (a few % of energy
    # variance) ride fp8 e3m4
    order = np.argsort(-np.abs(u))
    top, bot = order[:128], order[128:]

    def blockdiag(vals, dt):
        ust = np.zeros((128, NB, NB), dtype=dt)
        cast = (vals * USCALE).astype(dt)
        for bb in range(NB):
            ust[:, bb, bb] = cast
        return ust

    ubf = blockdiag(u[top], bf16)
    uf8 = blockdiag(u[bot], f8)
    id8 = np.eye(NB, dtype=bf16)
    iota = np.broadcast_to(np.arange(T, dtype=np.float32), (NB, T)).copy()
    lens_f = lens.astype(np.float32).reshape(B, 1)

    in_maps = []
    for c in range(NCORES):
        qc = q[c * NB : (c + 1) * NB]  # [NB, T, H]
        # [b, p, t]: per-batch DMAs, each partition's slice contiguous in HBM
        qbf = np.ascontiguousarray(qc[:, :, top].transpose(0, 2, 1)).astype(bf16)
        qf8 = (
            np.ascontiguousarray(qc[:, :, bot].transpose(0, 2, 1))
            .astype(f8)
            .reshape(NB // 2, 2, 128, T)
            .transpose(0, 2, 1, 3)
            .copy()
        )
        in_maps.append(
            {
                "qbf": qbf,
                "qf8": qf8,
                "ubf": ubf,
                "uf8": uf8,
                "id8": id8,
                "iota": iota,
                "lens": lens_f[c * NB : (c + 1) * NB],
            }
        )
    return in_maps


def _get_runner(reps=1):
    """Build (once per reps) a persistent sharded-jit runner over the 8 cores.

    Mirrors concourse.bass2jax.run_bass_via_pjrt's multi-core path, but caches
    the jitted executable so repeated calls skip retrace/recompile.  Used for
    benchmarking; the graded kernel() path goes through run_bass_kernel_spmd.
    """
    key = ("runner", reps)
    if key in _CACHE:
        return _CACHE[key]

    import jax
    from jax.sharding import Mesh, PartitionSpec
    from jax.experimental.shard_map import shard_map

    import concourse.mybir as mybir
    from concourse.bass2jax import (
        _bass_exec_p,
        install_neuronx_cc_hook,
        partition_id_tensor,
    )

    nc = _build_nc(reps)
    install_neuronx_cc_hook()

    partition_name = nc.partition_id_tensor.name if nc.partition_id_tensor else None
    in_names, out_names, out_avals, zero_outs = [], [], [], []
    for alloc in nc.m.functions[0].allocations:
        if not isinstance(alloc, mybir.MemoryLocationSet):
            continue
        name = alloc.memorylocations[0].name
        if alloc.kind == "ExternalInput":
            if name != partition_name:
                in_names.append(name)
        elif alloc.kind == "ExternalOutput":
            out_names.append(name)
            shape = tuple(alloc.tensor_shape)
            dtype = mybir.dt.np(alloc.dtype)
            out_avals.append(jax.core.ShapedArray(shape, dtype))
            zero_outs.append(np.zeros(shape, dtype))
    n_params = len(in_names)
    all_in_names = list(in_names) + list(out_names)
    if partition_name is not None:
        all_in_names.append(partition_name)

    def _body(*args):
        operands = list(args)
        if partition_name is not None:
            operands.append(partition_id_tensor())
        outs = _bass_exec_p.bind(
            *operands,
            out_avals=tuple(out_avals),
            in_names=tuple(all_in_names),
            out_names=tuple(out_names),
            lowering_input_output_aliases=(),
            sim_require_finite=True,
            sim_require_nnan=True,
            nc=nc,
        )
        return tuple(outs)

    devices = jax.devices()[:NCORES]
    mesh = Mesh(np.asarray(devices), ("core",))
    n_outs = len(out_names)
    in_specs = (PartitionSpec("core"),) * (n_params + n_outs)
    out_specs = (PartitionSpec("core"),) * n_outs
    sharded = jax.jit(
        shard_map(
            _body, mesh=mesh, in_specs=in_specs, out_specs=out_specs, check_rep=False
        ),
        donate_argnums=tuple(range(n_params, n_params + n_outs)),
        keep_unused=True,
    )

    def run(in_maps):
        concat_in = [
            np.concatenate([np.asarray(m[name]) for m in in_maps], axis=0)
            for name in in_names
        ]
        concat_zeros = [
            np.zeros((NCORES * z.shape[0], *z.shape[1:]), z.dtype) for z in zero_outs
        ]
        out_arrs = sharded(*concat_in, *concat_zeros)
        return {
            name: np.asarray(out_arrs[i]).reshape(NCORES * out_avals[i].shape[0], *out_avals[i].shape[1:])
            for i, name in enumerate(out_names)
        }

    _CACHE[("parts", reps)] = dict(
        sharded=sharded,
        in_names=in_names,
        out_names=out_names,
        out_avals=out_avals,
        zero_outs=zero_outs,
        mesh=mesh,
    )
    _CACHE[key] = run
    return run


def kernel(questions, questions_lens, W, b, v):
    """Full-input entry point: shards across the 8 NeuronCores, runs the Bass
    kernel via run_bass_kernel_spmd, gathers the full [64, 2048] output."""
    from concourse.bass_utils import run_bass_kernel_spmd

    if "nc" not in _CACHE:
        _CACHE["nc"] = _build_nc()
    in_maps = _prep_inputs(questions, questions_lens, W, b, v)
    res = run_bass_kernel_spmd(_CACHE["nc"], in_maps, list(range(NCORES)))
    return np.concatenate([r["out"] for r in res.results], axis=0)


# revision 34
# speedup vs baseline: 1.2986x; 1.2062x over previous
"""Trainium2 Bass kernel: masked attention-energy softmax.

Computes, for each batch row b:
    energy[b, t] = v . (W @ q[b, t] + bias)          (== q[b, t] . (W^T v) + bias . v)
    out[b]      = softmax(mask(energy[b]), axis=t)   with t >= len[b] masked to -1e10

Strategy
--------
* Data parallel over 8 NeuronCores, 8 batch rows per core.  W/b/v fold on
  host into u = W^T v (the bias.v constant cancels in softmax).
* Ragged packing: tokens beyond len[b] only need output 0, so there is no
  reason to stream them.  The host sorts the 64 rows by length and deals
  them round-robin across cores (slot s on core c gets global rank s*8+c);
  the program is then JIT-specialized per slot to a token budget Lr_s =
  ceil(max len in slot / 256)*256.  On the reference lens this streams 67%
  of the tokens.  Slot budgets are identical on every core, so one SPMD
  NEFF serves all 8.
* The per-token energy is a dot product -- TensorEngine work.  q is host-
  transposed (h on partitions) and split by |u_h| into mixed precision:
  top-128 components bf16, bottom-128 fp8 e3m4 (6.8% of sum u^2) -- 384 B
  per token.  Measured softmax error ~5e-3 (gate 2e-2).  Stationaries are
  scaled x64 so fp8 u values clear the e3m4 denormal floor; exp()
  compensates with scale=1/64.
* Each slot's energies land on PSUM partition s via a block-diagonal
  stationary (PE column strips force output bases to 0/32/64/96, so an M=8
  stationary is how 8 rows share one [8, 2048] PSUM tile).  Slot 0's budget
  is forced to the full T so its matmuls initialize every 256-wide PSUM
  region (start flag); shorter slots simply stop contributing -- their
  rows' uncovered columns stay whatever they were and are driven to -1e10
  by the mask matmuls, so exp() zeroes them.
* The ragged mask nm[s,t] = -1e10 * (t >= len[s]) is built once on DVE from
  a host iota + lens, then added in PSUM by full-width matmuls with an 8x8
  identity stationary (also closing every accumulation group).
* Tail on [8, 2048]: exp(E/64) + per-row accumulate on ScalarE straight
  from PSUM, reciprocal + scale on DVE, one 64 KB store.  No max-
  subtraction (E ~ N(0,1)); no cross-partition reduce; no gpsimd.
"""

import numpy as np

B, T, H = 64, 2048, 256
NCORES = 8
NB = B // NCORES  # batch rows (slots) per core
CT = 256  # matmul column tile (PSUM region width)
NEG = -1.0e10
USCALE = 64.0  # stationary pre-scale so fp8 u values stay normal
QBUFS = 10  # q tile pool depth (full-size tiles: 4 KB + 2 KB per partition)

_CACHE = {}


def _default_plan():
    return (T,) * NB


def _build_nc(reps=1, plan=None):
    """Build the per-core Bass program for the given slot token budgets.

    reps>1 statically unrolls the whole computation for benchmarking
    (marginal per-rep wall time isolates HW execution time from axon
    dispatch overhead); the graded path uses reps=1.
    """
    from contextlib import ExitStack

    import concourse.bacc as bacc
    import concourse.tile as tile
    from concourse import mybir

    if plan is None:
        plan = _CACHE.get("plan", _default_plan())
    plan = tuple(plan)
    assert plan[0] == T and all(x % CT == 0 and 0 < x <= T for x in plan)

    f32 = mybir.dt.float32
    bf16 = mybir.dt.bfloat16
    f8 = mybir.dt.float8e3
    nc = bacc.Bacc("TRN2", target_bir_lowering=False, debug=False)

    qbf_d = nc.dram_tensor("qbf", [NB, 128, T], bf16, kind="ExternalInput").ap()
    qf8_d = nc.dram_tensor("qf8", [NB, 128, T], f8, kind="ExternalInput").ap()
    ubf_d = nc.dram_tensor("ubf", [128, NB, NB], bf16, kind="ExternalInput").ap()
    uf8_d = nc.dram_tensor("uf8", [128, NB, NB], f8, kind="ExternalInput").ap()
    id8_d = nc.dram_tensor("id8", [NB, NB], bf16, kind="ExternalInput").ap()
    iota_d = nc.dram_tensor("iota", [NB, T], f32, kind="ExternalInput").ap()
    lens_d = nc.dram_tensor("lens", [NB, 1], f32, kind="ExternalInput").ap()
    out_d = nc.dram_tensor("out", [NB, T], f32, kind="ExternalOutput").ap()

    with tile.TileContext(nc) as tc, ExitStack() as ctx:
        singles = ctx.enter_context(tc.tile_pool(name="singles", bufs=1))
        qbpool = ctx.enter_context(tc.tile_pool(name="qbpool", bufs=QBUFS))
        qfpool = ctx.enter_context(tc.tile_pool(name="qfpool", bufs=QBUFS))
        ppool = ctx.enter_context(tc.tile_pool(name="ppool", bufs=2, space="PSUM"))
        spool = ctx.enter_context(tc.tile_pool(name="spool", bufs=2))

        ubf = singles.tile([128, NB, NB], bf16)
        nc.sync.dma_start(out=ubf, in_=ubf_d)
        uf8 = singles.tile([128, NB, NB], f8)
        nc.sync.dma_start(out=uf8, in_=uf8_d)
        id8 = singles.tile([NB, NB], bf16)
        nc.sync.dma_start(out=id8, in_=id8_d)
        iota_f = singles.tile([NB, T], f32)
        nc.sync.dma_start(out=iota_f, in_=iota_d)
        lens_sb = singles.tile([NB, 1], f32)
        nc.sync.dma_start(out=lens_sb, in_=lens_d)

        # nm[s, t] = NEG * (t >= len[s]); bf16 so it can ride a PE matmul
        nm = singles.tile([NB, T], bf16)
        nc.vector.tensor_scalar(
            out=nm,
            in0=iota_f,
            scalar1=lens_sb[:, 0:1],
            scalar2=NEG,
            op0=mybir.AluOpType.is_ge,
            op1=mybir.AluOpType.mult,
        )

        for _rep in range(reps):
            qbtiles, qftiles = [], []
            for s in range(NB):
                qb = qbpool.tile([128, T], bf16, tag="qb")
                nc.sync.dma_start(out=qb[:, 0 : plan[s]], in_=qbf_d[s][:, 0 : plan[s]])
                qbtiles.append(qb)
                qf = qfpool.tile([128, T], f8, tag="qf")
                nc.sync.dma_start(out=qf[:, 0 : plan[s]], in_=qf8_d[s][:, 0 : plan[s]])
                qftiles.append(qf)

            # USCALE * E[s, t] = sum_h q[s, t, h] * u[h], slot s on PSUM row s.
            # PSUM start/stop groups zero at bank granularity (512 f32), so
            # slot 0 (full width, start=True) uses 512-wide bank-aligned
            # tiles; shorter slots accumulate at the finer CT grid.
            ep = ppool.tile([NB, T], f32, tag="ep")
            for s in range(NB):
                ct = 512 if s == 0 else CT
                for nt in range(plan[s] // ct):
                    sl = slice(nt * ct, (nt + 1) * ct)
                    nc.tensor.matmul(
                        ep[:, sl],
                        ubf[:, s, :],
                        qbtiles[s][:, sl],
                        start=(s == 0),
                        stop=False,
                    )
                    nc.tensor.matmul(
                        ep[:, sl],
                        uf8[:, s, :],
                        qftiles[s][:, sl],
                        start=False,
                        stop=False,
                    )
            # E += nm, full width (identity stationary); also masks to -1e10
            # every column a short slot never wrote, and closes every group
            for nt in range(T // 512):
                nc.tensor.matmul(
                    ep[:, nt * 512 : (nt + 1) * 512],
                    id8,
                    nm[:, nt * 512 : (nt + 1) * 512],
                    start=False,
                    stop=True,
                )

            # expE[s, :] = exp(E[s, :] / USCALE), acc[s] = sum_t expE[s, t]
            # (masked slots hold ~ -1e10, exp -> 0 exactly)
            expE = spool.tile([NB, T], f32, tag="expE")
            acc = spool.tile([NB, 1], f32, tag="acc")
            nc.scalar.activation(
                out=expE,
                in_=ep,
                func=mybir.ActivationFunctionType.Exp,
                scale=1.0 / USCALE,
                accum_out=acc,
            )
            recip = spool.tile([NB, 1], f32, tag="recip")
            nc.vector.reciprocal(recip, acc)
            probs = spool.tile([NB, T], f32, tag="probs")
            nc.vector.tensor_scalar_mul(probs, expE, recip[:, 0:1])
            nc.sync.dma_start(out=out_d, in_=probs)

    nc.compile()
    return nc


def _make_plan(lens):
    """Sort rows by length (desc), deal round-robin across cores, and round
    each slot's budget up to the column-tile grid.  Slot 0 is pinned to the
    full T so its matmuls initialize every PSUM region."""
    order = np.argsort(-np.asarray(lens), kind="stable")
    dealt = order.reshape(NB, NCORES)  # dealt[s, c] = global row on core c slot s
    slot_max = np.asarray(lens)[dealt].max(axis=1)
    plan = [int(-(-int(m) // CT) * CT) for m in slot_max]
    plan[0] = T
    return tuple(plan), dealt


def _prep_inputs(questions, questions_lens, W, b, v):
    import ml_dtypes

    bf16 = ml_dtypes.bfloat16
    f8 = ml_dtypes.float8_e3m4
    q = np.asarray(questions, dtype=np.float32)
    lens = np.asarray(questions_lens)
    W = np.asarray(W, dtype=np.float32)
    v = np.asarray(v, dtype=np.float32)
    u = (W.T @ v).astype(np.float32)

    plan, dealt = _make_plan(lens)
    _CACHE["plan"] = plan
    _CACHE["dealt"] = dealt

    # split h by |u_h|: top 128 ride bf16, bottom 128 (a few % of energy
    # variance) ride fp8 e3m4
    order = np.argsort(-np.abs(u))
    top, bot = order[:128], order[128:]

    def blockdiag(vals, dt):
        ust = np.zeros((128, NB, NB), dtype=dt)
        cast = (vals * USCALE).astype(dt)
        for ss in range(NB):
            ust[:, ss, ss] = cast
        return ust

    ubf = blockdiag(u[top], bf16)
    uf8 = blockdiag(u[bot], f8)
    id8 = np.eye(NB, dtype=bf16)
    iota = np.broadcast_to(np.arange(T, dtype=np.float32), (NB, T)).copy()
    lens_f = lens.astype(np.float32)

    in_maps = []
    for c in range(NCORES):
        rows = dealt[:, c]  # global batch rows for this core, slot order
        qc = q[rows]  # [NB, T, H]
        # [s, p, t]: per-slot DMAs, each partition's slice contiguous in HBM
        qbf = np.ascontiguousarray(qc[:, :, top].transpose(0, 2, 1)).astype(bf16)
        qf8 = np.ascontiguousarray(qc[:, :, bot].transpose(0, 2, 1)).astype(f8)
        in_maps.append(
            {
                "qbf": qbf,
                "qf8": qf8,
                "ubf": ubf,
                "uf8": uf8,
                "id8": id8,
                "iota": iota,
                "lens": lens_f[rows].reshape(NB, 1),
            }
        )
    return in_maps


def _get_runner(reps=1):
    """Build (once per (reps, plan)) a persistent sharded-jit runner over the
    8 cores.  Mirrors concourse.bass2jax.run_bass_via_pjrt's multi-core path,
    but caches the jitted executable so repeated calls skip retrace/recompile.
    Used for benchmarking; the graded kernel() path uses run_bass_kernel_spmd.
    """
    plan = _CACHE.get("plan", _default_plan())
    key = ("runner", reps, plan)
    if key in _CACHE:
        return _CACHE[key]

    import jax
    from jax.sharding import Mesh, PartitionSpec
    from jax.experimental.shard_map import shard_map

    import concourse.mybir as mybir
    from concourse.bass2jax import (
        _bass_exec_p,
        install_neuronx_cc_hook,
        partition_id_tensor,
    )

    nc = _build_nc(reps, plan)
    install_neuronx_cc_hook()

    partition_name = nc.partition_id_tensor.name if nc.partition_id_tensor else None
    in_names, out_names, out_avals, zero_outs = [], [], [], []
    for alloc in nc.m.functions[0].allocations:
        if not isinstance(alloc, mybir.MemoryLocationSet):
            continue
        name = alloc.memorylocations[0].name
        if alloc.kind == "ExternalInput":
            if name != partition_name:
                in_names.append(name)
        elif alloc.kind == "ExternalOutput":
            out_names.append(name)
            shape = tuple(alloc.tensor_shape)
            dtype = mybir.dt.np(alloc.dtype)
            out_avals.append(jax.core.ShapedArray(shape, dtype))
            zero_outs.append(np.zeros(shape, dtype))
    n_params = len(in_names)
    all_in_names = list(in_names) + list(out_names)
    if partition_name is not None:
        all_in_names.append(partition_name)

    def _body(*args):
        operands = list(args)
        if partition_name is not None:
            operands.append(partition_id_tensor())
        outs = _bass_exec_p.bind(
            *operands,
            out_avals=tuple(out_avals),
            in_names=tuple(all_in_names),
            out_names=tuple(out_names),
            lowering_input_output_aliases=(),
            sim_require_finite=True,
            sim_require_nnan=True,
            nc=nc,
        )
        return tuple(outs)

    devices = jax.devices()[:NCORES]
    mesh = Mesh(np.asarray(devices), ("core",))
    n_outs = len(out_names)
    in_specs = (PartitionSpec("core"),) * (n_params + n_outs)
    out_specs = (PartitionSpec("core"),) * n_outs
    sharded = jax.jit(
        shard_map(
            _body, mesh=mesh, in_specs=in_specs, out_specs=out_specs, check_rep=False
        ),
        donate_argnums=tuple(range(n_params, n_params + n_outs)),
        keep_unused=True,
    )

    def run(in_maps):
        concat_in = [
            np.concatenate([np.asarray(m[name]) for m in in_maps], axis=0)
            for name in in_names
        ]
        concat_zeros = [
            np.zeros((NCORES * z.shape[0], *z.shape[1:]), z.dtype) for z in zero_outs
        ]
        out_arrs = sharded(*concat_in, *concat_zeros)
        return {
            name: np.asarray(out_arrs[i]).reshape(NCORES * out_avals[i].shape[0], *out_avals[i].shape[1:])
            for i, name in enumerate(out_names)
        }

    _CACHE[("parts", reps)] = dict(
        sharded=sharded,
        in_names=in_names,
        out_names=out_names,
        out_avals=out_avals,
        zero_outs=zero_outs,
        mesh=mesh,
    )
    _CACHE[key] = run
    return run


def kernel(questions, questions_lens, W, b, v):
    """Full-input entry point: shards across the 8 NeuronCores, runs the Bass
    kernel via run_bass_kernel_spmd, gathers the full [64, 2048] output."""
    from concourse.bass_utils import run_bass_kernel_spmd

    in_maps = _prep_inputs(questions, questions_lens, W, b, v)
    plan = _CACHE["plan"]
    nckey = ("nc", plan)
    if nckey not in _CACHE:
        _CACHE[nckey] = _build_nc(1, plan)
    res = run_bass_kernel_spmd(_CACHE[nckey], in_maps, list(range(NCORES)))
    dealt = _CACHE["dealt"]
    out = np.empty((B, T), dtype=np.float32)
    for c in range(NCORES):
        out[dealt[:, c]] = res.results[c]["out"]
    return out


# revision 35
# speedup vs baseline: 1.7230x; 1.3268x over previous
"""Trainium2 Bass kernel: masked attention-energy softmax.

Computes, for each batch row b:
    energy[b, t] = v . (W @ q[b, t] + bias)          (== q[b, t] . (W^T v) + bias . v)
    out[b]      = softmax(mask(energy[b]), axis=t)   with t >= len[b] masked to -1e10

Strategy
--------
* Data parallel over 8 NeuronCores, 8 batch rows per core.  W/b/v fold on
  host into u = W^T v (the bias.v constant cancels in softmax).
* Ragged packing: tokens beyond len[b] only need output 0, so there is no
  reason to stream them.  The host sorts the 64 rows by length and deals
  them round-robin across cores (slot s on core c gets global rank s*8+c);
  the program is then JIT-specialized per slot to a token budget Lr_s =
  ceil(max len in slot / 256)*256.  On the reference lens this streams 67%
  of the tokens.  Slot budgets are identical on every core, so one SPMD
  NEFF serves all 8.
* The per-token energy is a dot product -- TensorEngine work.  q is host-
  transposed (h on partitions) and split by |u_h| into mixed precision:
  top-128 components bf16, bottom-128 fp8 e3m4 (6.8% of sum u^2) -- 384 B
  per token.  Measured softmax error ~5e-3 (gate 2e-2).  Stationaries are
  scaled x64 so fp8 u values clear the e3m4 denormal floor; exp()
  compensates with scale=1/64.
* Each slot's energies land on PSUM partition s via a block-diagonal
  stationary (PE column strips force output bases to 0/32/64/96, so an M=8
  stationary is how 8 rows share one [8, 2048] PSUM tile).  Slot 0's budget
  is forced to the full T so its matmuls initialize every 256-wide PSUM
  region (start flag); shorter slots simply stop contributing -- their
  rows' uncovered columns stay whatever they were and are driven to -1e10
  by the mask matmuls, so exp() zeroes them.
* The ragged mask nm[s,t] = -1e10 * (t >= len[s]) is built once on DVE from
  a host iota + lens, then added in PSUM by full-width matmuls with an 8x8
  identity stationary (also closing every accumulation group).
* Tail on [8, 2048]: exp(E/64) + per-row accumulate on ScalarE straight
  from PSUM, reciprocal + scale on DVE, one 64 KB store.  No max-
  subtraction (E ~ N(0,1)); no cross-partition reduce; no gpsimd.
"""

import numpy as np

B, T, H = 64, 2048, 256
NCORES = 8
NB = B // NCORES  # batch rows (slots) per core
CT = 128  # matmul column tile for short slots (slot budget granularity)
NEG = -1.0e10
USCALE = 64.0  # stationary pre-scale so fp8 u values stay normal
QBUFS = 10  # q tile pool depth (full-size tiles: 4 KB + 2 KB per partition)

_CACHE = {}


def _default_plan():
    return (T,) * NB


def _build_nc(reps=1, plan=None):
    """Build the per-core Bass program for the given slot token budgets.

    reps>1 statically unrolls the whole computation for benchmarking
    (marginal per-rep wall time isolates HW execution time from axon
    dispatch overhead); the graded path uses reps=1.
    """
    from contextlib import ExitStack

    import concourse.bacc as bacc
    import concourse.tile as tile
    from concourse import mybir

    if plan is None:
        plan = _CACHE.get("plan", _default_plan())
    plan = tuple(plan)
    assert plan[0] == T and all(x % CT == 0 and 0 < x <= T for x in plan)

    f32 = mybir.dt.float32
    bf16 = mybir.dt.bfloat16
    f8 = mybir.dt.float8e3
    nc = bacc.Bacc("TRN2", target_bir_lowering=False, debug=False)

    qbf_d = nc.dram_tensor("qbf", [NB, 128, T], bf16, kind="ExternalInput").ap()
    qf8_d = nc.dram_tensor("qf8", [NB, 128, T], f8, kind="ExternalInput").ap()
    ubf_d = nc.dram_tensor("ubf", [128, NB, NB], bf16, kind="ExternalInput").ap()
    uf8_d = nc.dram_tensor("uf8", [128, NB, NB], f8, kind="ExternalInput").ap()
    id8_d = nc.dram_tensor("id8", [NB, NB], bf16, kind="ExternalInput").ap()
    iota_d = nc.dram_tensor("iota", [NB, T], f32, kind="ExternalInput").ap()
    lens_d = nc.dram_tensor("lens", [NB, 1], f32, kind="ExternalInput").ap()
    out_d = nc.dram_tensor("out", [NB, T], f32, kind="ExternalOutput").ap()

    with tile.TileContext(nc) as tc, ExitStack() as ctx:
        singles = ctx.enter_context(tc.tile_pool(name="singles", bufs=1))
        qbpool = ctx.enter_context(tc.tile_pool(name="qbpool", bufs=QBUFS))
        qfpool = ctx.enter_context(tc.tile_pool(name="qfpool", bufs=QBUFS))
        ppool = ctx.enter_context(tc.tile_pool(name="ppool", bufs=2, space="PSUM"))
        spool = ctx.enter_context(tc.tile_pool(name="spool", bufs=2))

        ubf = singles.tile([128, NB, NB], bf16)
        nc.sync.dma_start(out=ubf, in_=ubf_d)
        uf8 = singles.tile([128, NB, NB], f8)
        nc.sync.dma_start(out=uf8, in_=uf8_d)
        id8 = singles.tile([NB, NB], bf16)
        nc.sync.dma_start(out=id8, in_=id8_d)
        iota_f = singles.tile([NB, T], f32)
        nc.sync.dma_start(out=iota_f, in_=iota_d)
        lens_sb = singles.tile([NB, 1], f32)
        nc.sync.dma_start(out=lens_sb, in_=lens_d)

        # nm[s, t] = NEG * (t >= len[s]); bf16 so it can ride a PE matmul
        nm = singles.tile([NB, T], bf16)
        nc.vector.tensor_scalar(
            out=nm,
            in0=iota_f,
            scalar1=lens_sb[:, 0:1],
            scalar2=NEG,
            op0=mybir.AluOpType.is_ge,
            op1=mybir.AluOpType.mult,
        )

        for _rep in range(reps):
            qbtiles, qftiles = [], []
            for s in range(NB):
                qb = qbpool.tile([128, T], bf16, tag="qb")
                nc.sync.dma_start(out=qb[:, 0 : plan[s]], in_=qbf_d[s][:, 0 : plan[s]])
                qbtiles.append(qb)
                qf = qfpool.tile([128, T], f8, tag="qf")
                nc.sync.dma_start(out=qf[:, 0 : plan[s]], in_=qf8_d[s][:, 0 : plan[s]])
                qftiles.append(qf)

            # USCALE * E[s, t] = sum_h q[s, t, h] * u[h], slot s on PSUM row s.
            # PSUM start/stop groups zero at bank granularity (512 f32), so
            # slot 0 (full width, start=True) uses 512-wide bank-aligned
            # tiles; shorter slots accumulate at the finer CT grid.
            ep = ppool.tile([NB, T], f32, tag="ep")
            for s in range(NB):
                ct = 512 if s == 0 else CT
                for nt in range(plan[s] // ct):
                    sl = slice(nt * ct, (nt + 1) * ct)
                    nc.tensor.matmul(
                        ep[:, sl],
                        ubf[:, s, :],
                        qbtiles[s][:, sl],
                        start=(s == 0),
                        stop=False,
                    )
                    nc.tensor.matmul(
                        ep[:, sl],
                        uf8[:, s, :],
                        qftiles[s][:, sl],
                        start=False,
                        stop=False,
                    )
            # E += nm, full width (identity stationary); also masks to -1e10
            # every column a short slot never wrote, and closes every group
            for nt in range(T // 512):
                nc.tensor.matmul(
                    ep[:, nt * 512 : (nt + 1) * 512],
                    id8,
                    nm[:, nt * 512 : (nt + 1) * 512],
                    start=False,
                    stop=True,
                )

            # expE[s, :] = exp(E[s, :] / USCALE), acc[s] = sum_t expE[s, t]
            # (masked slots hold ~ -1e10, exp -> 0 exactly)
            expE = spool.tile([NB, T], f32, tag="expE")
            acc = spool.tile([NB, 1], f32, tag="acc")
            nc.scalar.activation(
                out=expE,
                in_=ep,
                func=mybir.ActivationFunctionType.Exp,
                scale=1.0 / USCALE,
                accum_out=acc,
            )
            recip = spool.tile([NB, 1], f32, tag="recip")
            nc.vector.reciprocal(recip, acc)
            probs = spool.tile([NB, T], f32, tag="probs")
            nc.vector.tensor_scalar_mul(probs, expE, recip[:, 0:1])
            nc.sync.dma_start(out=out_d, in_=probs)

    nc.compile()
    return nc


def _make_plan(lens):
    """Sort rows by length (desc), deal round-robin across cores, and round
    each slot's budget up to the column-tile grid.  Slot 0 is pinned to the
    full T so its matmuls initialize every PSUM region."""
    order = np.argsort(-np.asarray(lens), kind="stable")
    dealt = order.reshape(NB, NCORES)  # dealt[s, c] = global row on core c slot s
    slot_max = np.asarray(lens)[dealt].max(axis=1)
    plan = [int(-(-int(m) // CT) * CT) for m in slot_max]
    plan[0] = T
    return tuple(plan), dealt


def _prep_inputs(questions, questions_lens, W, b, v):
    import ml_dtypes

    bf16 = ml_dtypes.bfloat16
    f8 = ml_dtypes.float8_e3m4
    q = np.asarray(questions, dtype=np.float32)
    lens = np.asarray(questions_lens)
    W = np.asarray(W, dtype=np.float32)
    v = np.asarray(v, dtype=np.float32)
    u = (W.T @ v).astype(np.float32)

    plan, dealt = _make_plan(lens)
    _CACHE["plan"] = plan
    _CACHE["dealt"] = dealt

    # split h by |u_h|: top 128 ride bf16, bottom 128 (a few % of energy
    # variance) ride fp8 e3m4
    order = np.argsort(-np.abs(u))
    top, bot = order[:128], order[128:]

    def blockdiag(vals, dt):
        ust = np.zeros((128, NB, NB), dtype=dt)
        cast = (vals * USCALE).astype(dt)
        for ss in range(NB):
            ust[:, ss, ss] = cast
        return ust

    ubf = blockdiag(u[top], bf16)
    uf8 = blockdiag(u[bot], f8)
    id8 = np.eye(NB, dtype=bf16)
    iota = np.broadcast_to(np.arange(T, dtype=np.float32), (NB, T)).copy()
    lens_f = lens.astype(np.float32)

    in_maps = []
    for c in range(NCORES):
        rows = dealt[:, c]  # global batch rows for this core, slot order
        qc = q[rows]  # [NB, T, H]
        # [s, p, t]: per-slot DMAs, each partition's slice contiguous in HBM
        qbf = np.ascontiguousarray(qc[:, :, top].transpose(0, 2, 1)).astype(bf16)
        qf8 = np.ascontiguousarray(qc[:, :, bot].transpose(0, 2, 1)).astype(f8)
        in_maps.append(
            {
                "qbf": qbf,
                "qf8": qf8,
                "ubf": ubf,
                "uf8": uf8,
                "id8": id8,
                "iota": iota,
                "lens": lens_f[rows].reshape(NB, 1),
            }
        )
    return in_maps


def _get_runner(reps=1):
    """Build (once per (reps, plan)) a persistent sharded-jit runner over the
    8 cores.  Mirrors concourse.bass2jax.run_bass_via_pjrt's multi-core path,
    but caches the jitted executable so repeated calls skip retrace/recompile.
    Used for benchmarking; the graded kernel() path uses run_bass_kernel_spmd.
    """
    plan = _CACHE.get("plan", _default_plan())
    key = ("runner", reps, plan)
    if key in _CACHE:
        return _CACHE[key]

    import jax
    from jax.sharding import Mesh, PartitionSpec
    from jax.experimental.shard_map import shard_map

    import concourse.mybir as mybir
    from concourse.bass2jax import (
        _bass_exec_p,
        install_neuronx_cc_hook,
        partition_id_tensor,
    )

    nc = _build_nc(reps, plan)
    install_neuronx_cc_hook()

    partition_name = nc.partition_id_tensor.name if nc.partition_id_tensor else None
    in_names, out_names, out_avals, zero_outs = [], [], [], []
    for alloc in nc.m.functions[0].allocations:
        if not isinstance(alloc, mybir.MemoryLocationSet):
            continue
        name = alloc.memorylocations[0].name
        if alloc.kind == "ExternalInput":
            if name != partition_name:
                in_names.append(name)
        elif alloc.kind == "ExternalOutput":
            out_names.append(name)
            shape = tuple(alloc.tensor_shape)
            dtype = mybir.dt.np(alloc.dtype)
            out_avals.append(jax.core.ShapedArray(shape, dtype))
            zero_outs.append(np.zeros(shape, dtype))
    n_params = len(in_names)
    all_in_names = list(in_names) + list(out_names)
    if partition_name is not None:
        all_in_names.append(partition_name)

    def _body(*args):
        operands = list(args)
        if partition_name is not None:
            operands.append(partition_id_tensor())
        outs = _bass_exec_p.bind(
            *operands,
            out_avals=tuple(out_avals),
            in_names=tuple(all_in_names),
            out_names=tuple(out_names),
            lowering_input_output_aliases=(),
            sim_require_finite=True,
            sim_require_nnan=True,
            nc=nc,
        )
        return tuple(outs)

    devices = jax.devices()[:NCORES]
    mesh = Mesh(np.asarray(devices), ("core",))
    n_outs = len(out_names)
    in_specs = (PartitionSpec("core"),) * (n_params + n_outs)
    out_specs = (PartitionSpec("core"),) * n_outs
    sharded = jax.jit(
        shard_map(
            _body, mesh=mesh, in_specs=in_specs, out_specs=out_specs, check_rep=False
        ),
        donate_argnums=tuple(range(n_params, n_params + n_outs)),
        keep_unused=True,
    )

    def run(in_maps):
        concat_in = [
            np.concatenate([np.asarray(m[name]) for m in in_maps], axis=0)
            for name in in_names
        ]
        concat_zeros = [
            np.zeros((NCORES * z.shape[0], *z.shape[1:]), z.dtype) for z in zero_outs
        ]
        out_arrs = sharded(*concat_in, *concat_zeros)
        return {
            name: np.asarray(out_arrs[i]).reshape(NCORES * out_avals[i].shape[0], *out_avals[i].shape[1:])
            for i, name in enumerate(out_names)
        }

    _CACHE[("parts", reps)] = dict(
        sharded=sharded,
        in_names=in_names,
        out_names=out_names,
        out_avals=out_avals,
        zero_outs=zero_outs,
        mesh=mesh,
    )
    _CACHE[key] = run
    return run


def kernel(questions, questions_lens, W, b, v):
    """Full-input entry point: shards across the 8 NeuronCores, runs the Bass
    kernel via run_bass_kernel_spmd, gathers the full [64, 2048] output."""
    from concourse.bass_utils import run_bass_kernel_spmd

    in_maps = _prep_inputs(questions, questions_lens, W, b, v)
    plan = _CACHE["plan"]
    nckey = ("nc", plan)
    if nckey not in _CACHE:
        _CACHE[nckey] = _build_nc(1, plan)
    res = run_bass_kernel_spmd(_CACHE[nckey], in_maps, list(range(NCORES)))
    dealt = _CACHE["dealt"]
    out = np.empty((B, T), dtype=np.float32)
    for c in range(NCORES):
        out[dealt[:, c]] = res.results[c]["out"]
    return out


# revision 39
# speedup vs baseline: 1.7973x; 1.0431x over previous
"""Trainium2 Bass kernel: masked attention-energy softmax.

Computes, for each batch row b:
    energy[b, t] = v . (W @ q[b, t] + bias)          (== q[b, t] . (W^T v) + bias . v)
    out[b]      = softmax(mask(energy[b]), axis=t)   with t >= len[b] masked to -1e10

Strategy
--------
* Data parallel over 8 NeuronCores, 8 batch rows per core.  W/b/v fold on
  host into u = W^T v (the bias.v constant cancels in softmax).
* Ragged packing: tokens beyond len[b] only need output 0, so there is no
  reason to stream them.  The host sorts the 64 rows by length and deals
  them round-robin across cores (slot s on core c gets global rank s*8+c);
  the program is then JIT-specialized per slot to a token budget Lr_s =
  ceil(max len in slot / 256)*256.  On the reference lens this streams 67%
  of the tokens.  Slot budgets are identical on every core, so one SPMD
  NEFF serves all 8.
* The per-token energy is a dot product -- TensorEngine work.  q is host-
  transposed (h on partitions) and split by |u_h| into mixed precision:
  top-128 components bf16, bottom-128 fp8 e3m4 (6.8% of sum u^2) -- 384 B
  per token.  Measured softmax error ~5e-3 (gate 2e-2).  Stationaries are
  scaled x64 so fp8 u values clear the e3m4 denormal floor; exp()
  compensates with scale=1/64.
* Each slot's energies land on PSUM partition s via a block-diagonal
  stationary (PE column strips force output bases to 0/32/64/96, so an M=8
  stationary is how 8 rows share one [8, 2048] PSUM tile).  Slot 0's budget
  is forced to the full T so its matmuls initialize every 256-wide PSUM
  region (start flag); shorter slots simply stop contributing -- their
  rows' uncovered columns stay whatever they were and are driven to -1e10
  by the mask matmuls, so exp() zeroes them.
* The ragged mask nm[s,t] = -1e10 * (t >= len[s]) is built once on DVE from
  a host iota + lens, then added in PSUM by full-width matmuls with an 8x8
  identity stationary (also closing every accumulation group).
* Tail on [8, 2048]: exp(E/64) + per-row accumulate on ScalarE straight
  from PSUM, reciprocal + scale on DVE, one 64 KB store.  No max-
  subtraction (E ~ N(0,1)); no cross-partition reduce; no gpsimd.
"""

import numpy as np

B, T, H = 64, 2048, 256
NCORES = 8
NB = B // NCORES  # batch rows (slots) per core
CT = 128  # matmul column tile for short slots (slot budget granularity)
NEG = -1.0e10
USCALE = 64.0  # stationary pre-scale so fp8 u values stay normal
QBUFS = 10  # q tile pool depth (full-size tiles: 4 KB + 2 KB per partition)

_CACHE = {}


def _default_plan():
    return (T,) * NB


def _build_nc(reps=1, plan=None):
    """Build the per-core Bass program for the given slot token budgets.

    reps>1 statically unrolls the whole computation for benchmarking
    (marginal per-rep wall time isolates HW execution time from axon
    dispatch overhead); the graded path uses reps=1.
    """
    from contextlib import ExitStack

    import concourse.bacc as bacc
    import concourse.tile as tile
    from concourse import mybir

    if plan is None:
        plan = _CACHE.get("plan", _default_plan())
    plan = tuple(plan)
    assert plan[0] == T and all(x % CT == 0 and 0 < x <= T for x in plan)

    f32 = mybir.dt.float32
    bf16 = mybir.dt.bfloat16
    f8 = mybir.dt.float8e3
    nc = bacc.Bacc("TRN2", target_bir_lowering=False, debug=False)

    qbf_d = nc.dram_tensor("qbf", [NB, 128, T], bf16, kind="ExternalInput").ap()
    qf8_d = nc.dram_tensor("qf8", [NB, 128, T], f8, kind="ExternalInput").ap()
    ubf_d = nc.dram_tensor("ubf", [128, NB, NB], bf16, kind="ExternalInput").ap()
    uf8_d = nc.dram_tensor("uf8", [128, NB, NB], f8, kind="ExternalInput").ap()
    iota_d = nc.dram_tensor("iota", [NB, T], f32, kind="ExternalInput").ap()
    lens_d = nc.dram_tensor("lens", [NB, 1], f32, kind="ExternalInput").ap()
    out_d = nc.dram_tensor("out", [NB, T], f32, kind="ExternalOutput").ap()

    with tile.TileContext(nc) as tc, ExitStack() as ctx:
        singles = ctx.enter_context(tc.tile_pool(name="singles", bufs=1))
        qbpool = ctx.enter_context(tc.tile_pool(name="qbpool", bufs=QBUFS))
        qfpool = ctx.enter_context(tc.tile_pool(name="qfpool", bufs=QBUFS))
        ppool = ctx.enter_context(tc.tile_pool(name="ppool", bufs=2, space="PSUM"))
        spool = ctx.enter_context(tc.tile_pool(name="spool", bufs=2))

        ubf = singles.tile([128, NB, NB], bf16)
        nc.sync.dma_start(out=ubf, in_=ubf_d)
        uf8 = singles.tile([128, NB, NB], f8)
        nc.sync.dma_start(out=uf8, in_=uf8_d)
        iota_f = singles.tile([NB, T], f32)
        nc.sync.dma_start(out=iota_f, in_=iota_d)
        lens_sb = singles.tile([NB, 1], f32)
        nc.sync.dma_start(out=lens_sb, in_=lens_d)

        # nm[s, t] = USCALE * NEG * (t >= len[s]), pre-scaled to match the
        # USCALE-amplified energies in PSUM (added on DVE to keep the PE at
        # exactly 2 passes per streamed token)
        nm = singles.tile([NB, T], f32)
        nc.vector.tensor_scalar(
            out=nm,
            in0=iota_f,
            scalar1=lens_sb[:, 0:1],
            scalar2=NEG * USCALE,
            op0=mybir.AluOpType.is_ge,
            op1=mybir.AluOpType.mult,
        )

        for _rep in range(reps):
            qbtiles, qftiles = [], []
            for s in range(NB):
                qb = qbpool.tile([128, T], bf16, tag="qb")
                nc.sync.dma_start(out=qb[:, 0 : plan[s]], in_=qbf_d[s][:, 0 : plan[s]])
                qbtiles.append(qb)
                qf = qfpool.tile([128, T], f8, tag="qf")
                nc.sync.dma_start(out=qf[:, 0 : plan[s]], in_=qf8_d[s][:, 0 : plan[s]])
                qftiles.append(qf)

            # USCALE * E[s, t] = sum_h q[s, t, h] * u[h], slot s on PSUM row s.
            # PSUM start/stop groups zero at bank granularity (512 f32), so
            # slot 0 (full width, start=True) uses 512-wide bank-aligned
            # tiles; shorter slots accumulate at the finer CT grid.
            ep = ppool.tile([NB, T], f32, tag="ep")
            # per PSUM bank (512 f32 zero-region), the chronologically last
            # writer closes the accumulation group: the highest slot covering
            # any column of that bank, at its final tile inside the bank
            last_s = [max(s for s in range(NB) if plan[s] > 512 * k) for k in range(T // 512)]
            for s in range(NB):
                ct = 512 if s == 0 else CT
                for nt in range(plan[s] // ct):
                    sl = slice(nt * ct, (nt + 1) * ct)
                    bank = (nt * ct) // 512
                    stop = (s == last_s[bank]) and (
                        (nt + 1) * ct == min(plan[s], (bank + 1) * 512)
                    )
                    nc.tensor.matmul(
                        ep[:, sl],
                        ubf[:, s, :],
                        qbtiles[s][:, sl],
                        start=(s == 0),
                        stop=False,
                    )
                    nc.tensor.matmul(
                        ep[:, sl],
                        uf8[:, s, :],
                        qftiles[s][:, sl],
                        start=False,
                        stop=stop,
                    )

            # Em = E + nm on DVE: masks t >= len[s]; columns a short slot
            # never wrote hold 0 from slot 0's full-width start pass, and
            # 0 + -64e10 exps to 0 as required
            em = spool.tile([NB, T], f32, tag="em")
            nc.vector.tensor_add(em, ep, nm)

            # expE[s, :] = exp(Em[s, :] / USCALE), acc[s] = sum_t expE[s, t]
            expE = spool.tile([NB, T], f32, tag="expE")
            acc = spool.tile([NB, 1], f32, tag="acc")
            nc.scalar.activation(
                out=expE,
                in_=em,
                func=mybir.ActivationFunctionType.Exp,
                scale=1.0 / USCALE,
                accum_out=acc,
            )
            recip = spool.tile([NB, 1], f32, tag="recip")
            nc.vector.reciprocal(recip, acc)
            probs = spool.tile([NB, T], f32, tag="probs")
            nc.vector.tensor_scalar_mul(probs, expE, recip[:, 0:1])
            nc.sync.dma_start(out=out_d, in_=probs)

    nc.compile()
    return nc


def _make_plan(lens):
    """Sort rows by length (desc), deal round-robin across cores, and round
    each slot's budget up to the column-tile grid.  Slot 0 is pinned to the
    full T so its matmuls initialize every PSUM region."""
    order = np.argsort(-np.asarray(lens), kind="stable")
    dealt = order.reshape(NB, NCORES)  # dealt[s, c] = global row on core c slot s
    slot_max = np.asarray(lens)[dealt].max(axis=1)
    plan = [int(-(-int(m) // CT) * CT) for m in slot_max]
    plan[0] = T
    return tuple(plan), dealt


def _prep_inputs(questions, questions_lens, W, b, v):
    import ml_dtypes

    bf16 = ml_dtypes.bfloat16
    f8 = ml_dtypes.float8_e3m4
    q = np.asarray(questions, dtype=np.float32)
    lens = np.asarray(questions_lens)
    W = np.asarray(W, dtype=np.float32)
    v = np.asarray(v, dtype=np.float32)
    u = (W.T @ v).astype(np.float32)

    plan, dealt = _make_plan(lens)
    _CACHE["plan"] = plan
    _CACHE["dealt"] = dealt

    # split h by |u_h|: top 128 ride bf16, bottom 128 (a few % of energy
    # variance) ride fp8 e3m4
    order = np.argsort(-np.abs(u))
    top, bot = order[:128], order[128:]

    def blockdiag(vals, dt):
        ust = np.zeros((128, NB, NB), dtype=dt)
        cast = (vals * USCALE).astype(dt)
        for ss in range(NB):
            ust[:, ss, ss] = cast
        return ust

    ubf = blockdiag(u[top], bf16)
    uf8 = blockdiag(u[bot], f8)
    iota = np.broadcast_to(np.arange(T, dtype=np.float32), (NB, T)).copy()
    lens_f = lens.astype(np.float32)

    in_maps = []
    for c in range(NCORES):
        rows = dealt[:, c]  # global batch rows for this core, slot order
        qc = q[rows]  # [NB, T, H]
        # [s, p, t]: per-slot DMAs, each partition's slice contiguous in HBM
        qbf = np.ascontiguousarray(qc[:, :, top].transpose(0, 2, 1)).astype(bf16)
        qf8 = np.ascontiguousarray(qc[:, :, bot].transpose(0, 2, 1)).astype(f8)
        in_maps.append(
            {
                "qbf": qbf,
                "qf8": qf8,
                "ubf": ubf,
                "uf8": uf8,
                "iota": iota,
                "lens": lens_f[rows].reshape(NB, 1),
            }
        )
    return in_maps


def _get_runner(reps=1):
    """Build (once per (reps, plan)) a persistent sharded-jit runner over the
    8 cores.  Mirrors concourse.bass2jax.run_bass_via_pjrt's multi-core path,
    but caches the jitted executable so repeated calls skip retrace/recompile.
    Used for benchmarking; the graded kernel() path uses run_bass_kernel_spmd.
    """
    plan = _CACHE.get("plan", _default_plan())
    key = ("runner", reps, plan)
    if key in _CACHE:
        return _CACHE[key]

    import jax
    from jax.sharding import Mesh, PartitionSpec
    from jax.experimental.shard_map import shard_map

    import concourse.mybir as mybir
    from concourse.bass2jax import (
        _bass_exec_p,
        install_neuronx_cc_hook,
        partition_id_tensor,
    )

    nc = _build_nc(reps, plan)
    install_neuronx_cc_hook()

    partition_name = nc.partition_id_tensor.name if nc.partition_id_tensor else None
    in_names, out_names, out_avals, zero_outs = [], [], [], []
    for alloc in nc.m.functions[0].allocations:
        if not isinstance(alloc, mybir.MemoryLocationSet):
            continue
        name = alloc.memorylocations[0].name
        if alloc.kind == "ExternalInput":
            if name != partition_name:
                in_names.append(name)
        elif alloc.kind == "ExternalOutput":
            out_names.append(name)
            shape = tuple(alloc.tensor_shape)
            dtype = mybir.dt.np(alloc.dtype)
            out_avals.append(jax.core.ShapedArray(shape, dtype))
            zero_outs.append(np.zeros(shape, dtype))
    n_params = len(in_names)
    all_in_names = list(in_names) + list(out_names)
    if partition_name is not None:
        all_in_names.append(partition_name)

    def _body(*args):
        operands = list(args)
        if partition_name is not None:
            operands.append(partition_id_tensor())
        outs = _bass_exec_p.bind(
            *operands,
            out_avals=tuple(out_avals),
            in_names=tuple(all_in_names),
            out_names=tuple(out_names),
            lowering_input_output_aliases=(),
            sim_require_finite=True,
            sim_require_nnan=True,
            nc=nc,
        )
        return tuple(outs)

    devices = jax.devices()[:NCORES]
    mesh = Mesh(np.asarray(devices), ("core",))
    n_outs = len(out_names)
    in_specs = (PartitionSpec("core"),) * (n_params + n_outs)
    out_specs = (PartitionSpec("core"),) * n_outs
    sharded = jax.jit(
        shard_map(
            _body, mesh=mesh, in_specs=in_specs, out_specs=out_specs, check_rep=False
        ),
        donate_argnums=tuple(range(n_params, n_params + n_outs)),
        keep_unused=True,
    )

    def run(in_maps):
        concat_in = [
            np.concatenate([np.asarray(m[name]) for m in in_maps], axis=0)
            for name in in_names
        ]
        concat_zeros = [
            np.zeros((NCORES * z.shape[0], *z.shape[1:]), z.dtype) for z in zero_outs
        ]
        out_arrs = sharded(*concat_in, *concat_zeros)
        return {
            name: np.asarray(out_arrs[i]).reshape(NCORES * out_avals[i].shape[0], *out_avals[i].shape[1:])
            for i, name in enumerate(out_names)
        }

    _CACHE[("parts", reps)] = dict(
        sharded=sharded,
        in_names=in_names,
        out_names=out_names,
        out_avals=out_avals,
        zero_outs=zero_outs,
        mesh=mesh,
    )
    _CACHE[key] = run
    return run


def kernel(questions, questions_lens, W, b, v):
    """Full-input entry point: shards across the 8 NeuronCores, runs the Bass
    kernel via run_bass_kernel_spmd, gathers the full [64, 2048] output."""
    from concourse.bass_utils import run_bass_kernel_spmd

    in_maps = _prep_inputs(questions, questions_lens, W, b, v)
    plan = _CACHE["plan"]
    nckey = ("nc", plan)
    if nckey not in _CACHE:
        _CACHE[nckey] = _build_nc(1, plan)
    res = run_bass_kernel_spmd(_CACHE[nckey], in_maps, list(range(NCORES)))
    dealt = _CACHE["dealt"]
    out = np.empty((B, T), dtype=np.float32)
    for c in range(NCORES):
        out[dealt[:, c]] = res.results[c]["out"]
    return out
